# revision 7
# baseline (speedup 1.0000x reference)
"""Trainium2 Bass kernel for nn_AttnAdapter: GQA attention with RoPE,
region-based enhance/suppress score scaling, causal mask, o_proj.

Sharding: tensor-parallel over heads across 8 NeuronCores. Core d holds
q-heads 4d..4d+3 (wq rows), kv-head d (wk/wv rows), and wo columns
512d..512(d+1). Each core computes a full [S, D] partial of the output;
the host sums the 8 partials (the TP all-reduce, done at unshard time).

All on-chip matmuls run in float32r (full PE rate for N>=512) over
transposed layouts so no on-device transposes are needed except V
(16 PE 128x128 transposes).
"""

import math

import numpy as np

# ---- problem constants (hardcoded; kernel.py must be self-contained) ----
S = 2048          # sequence length
D = 4096          # model dim
HD = 128          # head dim
NCORES = 8
QH = 4            # q heads per core
SYS_LEN, IMG_LEN = 35, 576
BOUND = SYS_LEN + IMG_LEN          # 611
ENH, SUP = 1.5, 0.5
ROPE_BASE = 10000.0

J = 4             # sq tiles of 512
NSK = 16          # sk tiles of 128
DCH = 32          # D chunks of 128
F32 = None        # set after mybir import

_CACHE = {}


def _host_constants():
    """cosT/sinT [128,S], rotation matmul lhsT, identity, causal masks,
    key-scale columns, ones vectors."""
    inv_freq = 1.0 / (ROPE_BASE ** (np.arange(0, HD, 2, dtype=np.float32) / HD))
    pos = np.arange(S, dtype=np.float32)
    freqs = pos[:, None] * inv_freq[None, :]              # [S, 64]
    emb = np.concatenate([freqs, freqs], axis=-1)         # [S, 128]
    cosT = np.ascontiguousarray(np.cos(emb).T.astype(np.float32))  # [128, S]
    sinT = np.ascontiguousarray(np.sin(emb).T.astype(np.float32))

    # rotate_half as a matmul: rot = R @ q (in [hd, s] layout).
    # matmul(out, lhsT, rhs) = lhsT.T @ rhs, so feed RT = R.T.
    RT = np.zeros((HD, HD), dtype=np.float32)
    half = HD // 2
    for c in range(half):
        RT[c + half, c] = -1.0      # rot[c] = -q[c+64]
    for c in range(half, HD):
        RT[c - half, c] = 1.0       # rot[c] = q[c-64]

    ident = np.eye(HD, dtype=np.float32)

    # Diagonal-tile causal masks, T layout [sk 128, sq 512]:
    # tile (i=4j+delta, j): valid (keep) iff sq >= sk  <=>  f >= 128*delta + p
    masks = np.zeros((HD, 4 * 512), dtype=np.float32)
    p = np.arange(128)[:, None]
    f = np.arange(512)[None, :]
    for delta in range(4):
        masks[:, delta * 512:(delta + 1) * 512] = (f >= 128 * delta + p)

    # key_scale in partition layout per sk-tile: ksT[p, i] = scale(128*i+p)
    kpos = np.arange(S)
    key_scale = np.where(kpos < SYS_LEN, SUP,
                         np.where(kpos < BOUND, ENH, 1.0)).astype(np.float32)
    ksT = np.ascontiguousarray(key_scale[:5 * 128].reshape(5, 128).T)  # [128, 5]

    ones_col = np.ones((HD, 1), dtype=np.float32)
    ones_row = np.ones((1, HD), dtype=np.float32)
    return dict(cosT=cosT, sinT=sinT, rmat=RT, ident=ident, masks=masks,
                ksT=ksT, ones_col=ones_col, ones_row=ones_row)


def _build_bass():
    import concourse.bass as bass
    import concourse.mybir as mybir
    from concourse.tile import TileContext
    from contextlib import ExitStack

    f32 = mybir.dt.float32
    f32r = mybir.dt.float32r

    nc = bass.Bass()
    xT = nc.dram_tensor("xT", [D, S], f32r, kind="ExternalInput")
    wqT = nc.dram_tensor("wqT", [D, QH * HD], f32r, kind="ExternalInput")
    wkT = nc.dram_tensor("wkT", [D, HD], f32r, kind="ExternalInput")
    wvT = nc.dram_tensor("wvT", [D, HD], f32r, kind="ExternalInput")
    woT = nc.dram_tensor("woT", [QH * HD, D], f32r, kind="ExternalInput")
    cosT_d = nc.dram_tensor("cosT", [HD, S], f32, kind="ExternalInput")
    sinT_d = nc.dram_tensor("sinT", [HD, S], f32, kind="ExternalInput")
    rmat_d = nc.dram_tensor("rmat", [HD, HD], f32r, kind="ExternalInput")
    ident_d = nc.dram_tensor("ident", [HD, HD], f32, kind="ExternalInput")
    masks_d = nc.dram_tensor("masks", [HD, 4 * 512], f32, kind="ExternalInput")
    ksT_d = nc.dram_tensor("ksT", [HD, 5], f32, kind="ExternalInput")
    onesc_d = nc.dram_tensor("ones_col", [HD, 1], f32r, kind="ExternalInput")
    onesr_d = nc.dram_tensor("ones_row", [1, HD], f32r, kind="ExternalInput")
    out = nc.dram_tensor("out", [S, D], f32, kind="ExternalOutput")

    EXP = mybir.ActivationFunctionType.Exp

    with TileContext(nc) as tc, ExitStack() as ctx:
        const = ctx.enter_context(tc.tile_pool(name="const", bufs=1))
        cosT = const.tile([HD, S], f32)
        nc.sync.dma_start(cosT[:], cosT_d[:, :])
        sinT = const.tile([HD, S], f32)
        nc.sync.dma_start(sinT[:], sinT_d[:, :])
        rmat = const.tile([HD, HD], f32r)
        nc.sync.dma_start(rmat[:], rmat_d[:, :])
        ident = const.tile([HD, HD], f32)
        nc.sync.dma_start(ident[:], ident_d[:, :])
        masks = const.tile([HD, 4 * 512], f32)
        nc.sync.dma_start(masks[:], masks_d[:, :])
        ksT = const.tile([HD, 5], f32)
        nc.sync.dma_start(ksT[:], ksT_d[:, :])
        ones_col = const.tile([HD, 1], f32r)
        nc.sync.dma_start(ones_col[:], onesc_d[:, :])
        ones_row = const.tile([1, HD], f32r)
        nc.sync.dma_start(ones_row[:], onesr_d[:, :])

        persist = ctx.enter_context(tc.tile_pool(name="persist", bufs=1))
        qrot = [persist.tile([HD, S], f32r, name=f"qrot{m}") for m in range(QH)]
        krot = persist.tile([HD, S], f32r)
        vnat = persist.tile([HD, NSK * HD], f32r)  # tile i at cols i*128
        attn = [persist.tile([HD, S], f32r, name=f"attn{h}") for h in range(QH)]

        # ---------------- Phase A: projections + RoPE + V transpose --------
        with tc.tile_pool(name="xw", bufs=4) as xw, \
             tc.tile_pool(name="accp", bufs=1, space="PSUM") as accp, \
             tc.tile_pool(name="ropep", bufs=2, space="PSUM") as ropep, \
             tc.tile_pool(name="stage", bufs=3) as stage:
            for j in range(J):
                sq = slice(j * 512, (j + 1) * 512)
                accs = [accp.tile([128, 512], f32, name=f"acc{m}") for m in range(6)]
                for d in range(DCH):
                    dd = slice(d * 128, (d + 1) * 128)
                    xt = xw.tile([128, 512], f32r, tag="xt")
                    nc.sync.dma_start(xt[:], xT[dd, sq])
                    wq_t = xw.tile([128, 512], f32r, tag="wq")
                    nc.sync.dma_start(wq_t[:], wqT[dd, :])
                    wkv_t = xw.tile([128, 256], f32r, tag="wkv")
                    nc.sync.dma_start(wkv_t[:, 0:128], wkT[dd, :])
                    nc.sync.dma_start(wkv_t[:, 128:256], wvT[dd, :])
                    st = (d == 0)
                    sp = (d == DCH - 1)
                    for m in range(QH):
                        nc.tensor.matmul(accs[m][:], wq_t[:, m * 128:(m + 1) * 128],
                                         xt[:], start=st, stop=sp)
                    nc.tensor.matmul(accs[4][:], wkv_t[:, 0:128], xt[:],
                                     start=st, stop=sp)
                    nc.tensor.matmul(accs[5][:], wkv_t[:, 128:256], xt[:],
                                     start=st, stop=sp)

                # RoPE for q tiles and k tile
                for m in range(5):
                    dst = qrot[m][:, sq] if m < QH else krot[:, sq]
                    q_sb = stage.tile([128, 512], f32r, tag="q_sb")
                    nc.scalar.copy(q_sb[:], accs[m][:])
                    rot_ps = ropep.tile([128, 512], f32, tag="rope_ps")
                    nc.tensor.matmul(rot_ps[:], rmat[:], q_sb[:],
                                     start=True, stop=True)
                    t1 = stage.tile([128, 512], f32, tag="t1")
                    nc.vector.tensor_mul(t1[:], accs[m][:], cosT[:, sq])
                    t2 = stage.tile([128, 512], f32, tag="t2")
                    nc.vector.tensor_mul(t2[:], rot_ps[:], sinT[:, sq])
                    nc.vector.tensor_add(dst, t1[:], t2[:])

                # V: copy to SBUF, transpose 128x128 blocks into vnat
                v_sb = stage.tile([128, 512], f32, tag="v_sb")
                nc.scalar.copy(v_sb[:], accs[5][:])
                for b in range(4):
                    i = 4 * j + b
                    vt_ps = ropep.tile([128, 512], f32, tag="rope_ps")
                    nc.tensor.transpose(vt_ps[:, 0:128],
                                        v_sb[:, b * 128:(b + 1) * 128], ident[:])
                    nc.vector.tensor_copy(vnat[:, i * 128:(i + 1) * 128],
                                          vt_ps[:, 0:128])

        # ---------------- Phase B: attention ------------------------------
        with tc.tile_pool(name="att_sb", bufs=4) as att_sb, \
             tc.tile_pool(name="sp", bufs=2, space="PSUM") as sp, \
             tc.tile_pool(name="avp", bufs=2, space="PSUM") as avp, \
             tc.tile_pool(name="dnp", bufs=2, space="PSUM") as dnp, \
             tc.tile_pool(name="rbp", bufs=2, space="PSUM") as rbp, \
             tc.tile_pool(name="nrm", bufs=2) as nrm:
            for j in range(J):
                sq = slice(j * 512, (j + 1) * 512)
                ni = 4 * j + 4            # sk tiles 0..4j+3 are live
                for h in range(QH):
                    acc_av = avp.tile([128, 512], f32, tag="av")
                    acc_dn = dnp.tile([1, 512], f32, tag="dn")
                    for i in range(ni):
                        s_ps = sp.tile([128, 512], f32, tag="s")
                        nc.tensor.matmul(s_ps[:], krot[:, i * 128:(i + 1) * 128],
                                         qrot[h][:, sq], start=True, stop=True)
                        if i < 5:
                            # region enhance/suppress for sq >= BOUND
                            c0 = 0 if j >= 2 else (BOUND - 512 if j == 1 else None)
                            if c0 is not None:
                                nc.vector.tensor_scalar_mul(
                                    s_ps[:, c0:512], s_ps[:, c0:512],
                                    ksT[:, i:i + 1])
                        e_sb = att_sb.tile([128, 512], f32r, tag="e")
                        nc.scalar.activation(e_sb[:], s_ps[:], EXP)
                        delta = i - 4 * j
                        if delta >= 0:
                            nc.vector.tensor_mul(
                                e_sb[:], e_sb[:],
                                masks[:, delta * 512:(delta + 1) * 512])
                        st = (i == 0)
                        sp_l = (i == ni - 1)
                        nc.tensor.matmul(acc_dn[:], ones_col[:], e_sb[:],
                                         start=st, stop=sp_l)
                        nc.tensor.matmul(acc_av[:], vnat[:, i * 128:(i + 1) * 128],
                                         e_sb[:], start=st, stop=sp_l)
                    den_sb = nrm.tile([1, 512], f32, tag="den")
                    nc.vector.tensor_copy(den_sb[:], acc_dn[:])
                    rec_sb = nrm.tile([1, 512], f32r, tag="rec")
                    with nc.allow_low_precision(
                            reason="softmax denom reciprocal feeds f32r bcast"):
                        nc.vector.reciprocal(rec_sb[:], den_sb[:])
                    rb_ps = rbp.tile([128, 512], f32, tag="rb")
                    nc.tensor.matmul(rb_ps[:], ones_row[:], rec_sb[:],
                                     start=True, stop=True)
                    rb_sb = att_sb.tile([128, 512], f32, tag="rb_sb")
                    nc.vector.tensor_copy(rb_sb[:], rb_ps[:])
                    nc.vector.tensor_mul(attn[h][:, sq], acc_av[:], rb_sb[:])

        # ---------------- Phase C: o_proj ---------------------------------
        with tc.tile_pool(name="wo_sb", bufs=1) as wo_sb, \
             tc.tile_pool(name="op", bufs=4, space="PSUM") as op, \
             tc.tile_pool(name="ost", bufs=4) as ost:
            wo_t = [wo_sb.tile([128, D], f32r, name=f"wo{h}") for h in range(QH)]
            for h in range(QH):
                nc.sync.dma_start(wo_t[h][:], woT[h * 128:(h + 1) * 128, :])
            for t in range(NSK):
                ts_ = slice(t * 128, (t + 1) * 128)
                for n in range(8):
                    oc = slice(n * 512, (n + 1) * 512)
                    o_ps = op.tile([128, 512], f32, tag="o")
                    for h in range(QH):
                        nc.tensor.matmul(o_ps[:], attn[h][:, ts_],
                                         wo_t[h][:, oc],
                                         start=(h == 0), stop=(h == QH - 1))
                    o_sb = ost.tile([128, 512], f32, tag="o_sb")
                    nc.any.tensor_copy(o_sb[:], o_ps[:])
                    nc.sync.dma_start(out[ts_, oc], o_sb[:])

    # Split multi-wait instructions (self-loading f32r matmuls allow only
    # one sync wait) onto standalone EventSemaphore instructions.
    import bass_rust
    bass_rust.generate_event_semaphores(nc)
    return nc


def _get_compiled():
    if "nc" not in _CACHE:
        _CACHE["nc"] = _build_bass()
        _CACHE["const"] = _host_constants()
    return _CACHE["nc"], _CACHE["const"]


def kernel(hidden_states, wq, wk, wv, wo, _trace=False):
    from concourse.bass_utils import run_bass_kernel_spmd

    nc, cst = _get_compiled()

    x = np.asarray(hidden_states, dtype=np.float32).reshape(S, D)
    xT = np.ascontiguousarray(x.T)
    wq = np.asarray(wq, dtype=np.float32)
    wk = np.asarray(wk, dtype=np.float32)
    wv = np.asarray(wv, dtype=np.float32)
    wo = np.asarray(wo, dtype=np.float32)
    scale = 1.0 / math.sqrt(HD)

    in_maps = []
    for d in range(NCORES):
        wq_d = wq[d * QH * HD:(d + 1) * QH * HD] * scale      # [512, D]
        in_maps.append({
            "xT": xT,
            "wqT": np.ascontiguousarray(wq_d.T),
            "wkT": np.ascontiguousarray(wk[d * HD:(d + 1) * HD].T),
            "wvT": np.ascontiguousarray(wv[d * HD:(d + 1) * HD].T),
            "woT": np.ascontiguousarray(wo[:, d * QH * HD:(d + 1) * QH * HD].T),
            "cosT": cst["cosT"], "sinT": cst["sinT"],
            "rmat": cst["rmat"], "ident": cst["ident"],
            "masks": cst["masks"], "ksT": cst["ksT"],
            "ones_col": cst["ones_col"], "ones_row": cst["ones_row"],
        })

    res = run_bass_kernel_spmd(nc, in_maps, core_ids=list(range(NCORES)),
                               trace=_trace)
    acc = res.results[0]["out"].astype(np.float64)
    for d in range(1, NCORES):
        acc += res.results[d]["out"]
    outp = acc.astype(np.float32).reshape(1, S, D)
    if _trace:
        _CACHE["last_results"] = res
    return outp
